# revision 17
# baseline (speedup 1.0000x reference)
"""Cross-attention Trainium2 kernel (8 NeuronCores, SPMD), v5.

Reference computation (per full batch):
  q = x @ Wq + bq;  k = enc @ Wk + bk;  v = enc @ Wv + bv
  att = softmax((q k^T) / sqrt(D));  y = (att v) @ Wo + bo

Sharding: B(=4) x T-half(=2) -> 8 cores. Each core handles one batch
element and half of the 2048 query tokens, all 16 heads, and writes
out[b, t_half] directly.

Design (vs the 415us f32r v0 baseline; measured ~295-308us, paired-min
~251us):
  * All matmul operands bf16 (1 cycle/row PE rate, half the LDWEIGHTS
    bytes of f32r, rel err ~3e-3 vs the 2e-2 budget).
  * Inputs arrive host-side pre-transposed and pre-cast:
      xT/encT  [128, 8, 1024] bf16   ([p, c, t] = x[t, c*128+p])
      wq/wk/wv/wo [128, 8, 1024] bf16 ([p, kc, n] = W[kc*128+p, n])
    eliminating all on-device PE transposes and their DVE copies.
  * kz [128, 16, 1024]: zero-padded per-head K so every matmul in the
    program runs the same (128,128) PE tile config -- mixing K=64
    scores with K=128 matmuls forces PE tile-config switches that
    expose every LDWEIGHTS (~40 ns/matmul cadence penalty). The pads
    are zeroed once by the idle Pool engine; K-proj writes only the
    in-head halves, so the zeros persist.
  * v is built directly in the att@v lhsT layout vS2[u] [128, 2, 16*65]
    (65-column groups per head: [v_h | ones]); the ones column makes
    psum row 64 of ya the softmax denominator for free, and the
    attention loop needs no per-head lhsT staging at all.
  * Softmax: exp on ACT (psum -> bf16, fused 1/sqrt(D) scale);
    normalization = DVE copy of the denominator row to SBUF (custom-DVE
    ops silently misread PSUM on HW), reciprocal_approx_fast, gpsimd
    partition_broadcast (source must start at partition 0 -- BIR
    verifier rejects other base partitions), one DVE multiply into yT.
  * Software-pipelined schedule: upfront K proj + V(heads 0-7 half) +
    Q(chunk 0) while chunked DMAs land in need-order across both HWDGE
    queues (sync: encT, wv-hi, xT; scalar: wk, wv-lo, wq, wo); the
    remaining Q/V projection groups and the first-half out-projection
    (y chunks 0-3, ready after head 7) are interleaved as filler work
    into the exp-paced attention heads; only the second-half out
    contraction remains as serial tail.
  * PSUM: scores 2x[128,1024] (4 banks) + ya 2x[65,512] (2) + shared
    projection/out psum 2x[128,512] (2) = exactly 8 banks.

Engine budget per core: PE ~245us busy (the bottleneck: 1024 matmuls
at ~220-240ns cadence incl LDWEIGHTS), ACT ~155us (128 exps of
[128,1024] at ~1.11us), DVE ~130us, Pool ~50us.
"""

import sys

sys.path.insert(0, "/opt/trn_rl_repo")

import numpy as np

import concourse.bass as bass  # noqa: E402,F401
import concourse.tile as tile  # noqa: E402
from concourse import bacc, mybir  # noqa: E402

F32 = mybir.dt.float32
BF16 = mybir.dt.bfloat16
AF = mybir.ActivationFunctionType

P = 128          # partitions
TOK = 1024       # query tokens per core
T2 = 1024        # kv sequence length
C = 1024         # embed dim
H = 16           # heads
D = 64           # head dim
NCH = C // P     # 8 channel chunks
NS = T2 // P     # 8 kv-position chunks
TN = 512         # matmul moving-dim tile
G = D + 1        # v-group stride in vS2 (64 v cols + ones col)
SCALE = 1.0 / np.sqrt(D)

N_CORES = 8
B_FULL, T_FULL = 4, 2048


def build_program(loop_iters=None, debug=False):
    """loop_iters: if set, wrap the body in a For_i hardware loop (timing)."""
    nc = bacc.Bacc("TRN2", target_bir_lowering=False, debug=False,
                   num_devices=N_CORES)

    aps = {}
    for name in ("xT", "encT", "wq", "wk", "wv", "wo"):
        aps[name] = nc.dram_tensor(name, [P, NCH, 1024], BF16,
                                   kind="ExternalInput").ap()
    for name in ("bqc", "bkc"):
        aps[name] = nc.dram_tensor(name, [P, NCH], F32,
                                   kind="ExternalInput").ap()
    for name in ("bv", "bo"):
        aps[name] = nc.dram_tensor(name, [C], F32, kind="ExternalInput").ap()
    out = nc.dram_tensor("out", [TOK, C], F32, kind="ExternalOutput").ap()

    dbg = None
    if debug:
        dbg = {}
        for name, shape, dt in (
                ("d_kT", [P, H, T2], BF16), ("d_qT", [P, NCH, TOK], BF16),
                ("d_yT", [P, NCH, TOK], BF16),
                ("d_vS0", [P, 2, H * G], BF16),
                ("d_px0", [P, TOK], BF16), ("d_px7", [P, TOK], BF16),
                ("d_ya0", [D + 1, TN], F32), ("d_ya1", [D + 1, TN], F32)):
            dbg[name] = nc.dram_tensor(name, shape, dt,
                                       kind="ExternalOutput").ap()

    with tile.TileContext(nc) as tc:
        if loop_iters is not None:
            with tc.For_i(0, loop_iters, 1):
                _emit(nc, tc, aps, out)
        else:
            _emit(nc, tc, aps, out, dbg)

    nc.compile()
    return nc


def _row(ap):
    return ap.rearrange("(a c) -> a c", a=1)


def _emit(nc, tc, aps, out, dbg=None):
    from contextlib import ExitStack

    with ExitStack() as S:
        pIn = S.enter_context(tc.tile_pool(name="pIn", bufs=1))

        # ---- persistent tiles
        wo = pIn.tile([P, NCH, C], BF16, tag="wo", name="wo")
        bo_row = pIn.tile([1, C], F32, tag="bo_row", name="bo_row")
        kz = pIn.tile([P, H, T2], BF16, tag="kz", name="kz")
        qT = pIn.tile([P, NCH, TOK], BF16, tag="qT", name="qT")
        yT = pIn.tile([P, NCH, TOK], BF16, tag="yT", name="yT")
        vS2 = [pIn.tile([P, 2, H * G], BF16, tag=f"vS2_{u}", name=f"vS2_{u}")
               for u in range(NS // 2)]

        psP = S.enter_context(tc.tile_pool(name="psP", bufs=2, space="PSUM"))
        psS = S.enter_context(tc.tile_pool(name="psS", bufs=2, space="PSUM"))
        psY = S.enter_context(tc.tile_pool(name="psY", bufs=2, space="PSUM"))
        pPx = S.enter_context(tc.tile_pool(name="pPx", bufs=3))
        pRc = S.enter_context(tc.tile_pool(name="pRc", bufs=2))
        pBc = S.enter_context(tc.tile_pool(name="pBc", bufs=2))

        state = {}

        def open_proj_scope(S2):
            # DMA priority: K-proj inputs first on both HWDGE queues, then
            # wv split across both, then Q-proj inputs, wo last.
            pTmp = S2.enter_context(tc.tile_pool(name="pTmp", bufs=1))
            bkc = pTmp.tile([P, NCH], F32, tag="bkc", name="bkc")
            nc.sync.dma_start(out=bkc, in_=aps["bkc"])
            bqc = pTmp.tile([P, NCH], F32, tag="bqc", name="bqc")
            nc.sync.dma_start(out=bqc, in_=aps["bqc"])
            bv_row = pTmp.tile([1, C], F32, tag="bv_row", name="bv_row")
            nc.sync.dma_start(out=bv_row, in_=_row(aps["bv"]))
            nc.sync.dma_start(out=bo_row, in_=_row(aps["bo"]))

            encT = pTmp.tile([P, NCH, T2], BF16, tag="encT", name="encT")
            wk = pTmp.tile([P, NCH, C], BF16, tag="wk", name="wk")
            wv = pTmp.tile([P, NCH, C], BF16, tag="wv", name="wv")
            wq = pTmp.tile([P, NCH, C], BF16, tag="wq", name="wq")
            xT = pTmp.tile([P, NCH, TOK], BF16, tag="xT", name="xT")
            for kc in range(NCH):
                nc.sync.dma_start(out=encT[:, kc, :], in_=aps["encT"][:, kc, :])
                nc.scalar.dma_start(out=wk[:, kc, :], in_=aps["wk"][:, kc, :])
            for kc in range(NCH):
                q = nc.sync if kc >= 4 else nc.scalar
                q.dma_start(out=wv[:, kc, :], in_=aps["wv"][:, kc, :])
            for kc in range(NCH):
                nc.sync.dma_start(out=xT[:, kc, :], in_=aps["xT"][:, kc, :])
            for kc in range(NCH):
                nc.scalar.dma_start(out=wq[:, kc, :], in_=aps["wq"][:, kc, :])
            nc.scalar.dma_start(out=wo, in_=aps["wo"])

            bvb = pTmp.tile([P, C], F32, tag="bvb", name="bvb")
            nc.gpsimd.partition_broadcast(bvb, bv_row)
            state.update(encT=encT, wk=wk, wv=wv, wq=wq, xT=xT,
                         bkc=bkc, bqc=bqc, bvb=bvb)

        def k_group(c, tn):
            ps = psP.tile([P, TN], F32, tag="pp", name="psK")
            for kc in range(NCH):
                nc.tensor.matmul(
                    ps, state["wk"][:, kc, c * P:(c + 1) * P],
                    state["encT"][:, kc, tn * TN:(tn + 1) * TN],
                    start=(kc == 0), stop=(kc == NCH - 1))
            bkc = state["bkc"]
            ts = slice(tn * TN, (tn + 1) * TN)
            nc.vector.tensor_scalar_add(kz[0:D, 2 * c, ts], ps[0:D, :],
                                        bkc[0:D, c:c + 1])
            nc.vector.tensor_scalar_add(kz[D:P, 2 * c + 1, ts], ps[D:P, :],
                                        bkc[D:P, c:c + 1])

        def kq_group(w, src, dst, bcol, c, tn):
            ps = psP.tile([P, TN], F32, tag="pp", name="psP")
            for kc in range(NCH):
                nc.tensor.matmul(
                    ps, w[:, kc, c * P:(c + 1) * P],
                    src[:, kc, tn * TN:(tn + 1) * TN],
                    start=(kc == 0), stop=(kc == NCH - 1))
            nc.vector.tensor_scalar_add(dst[:, c, tn * TN:(tn + 1) * TN], ps,
                                        bcol[:, c:c + 1])

        def v_group(sc, nn):
            u, j = sc // 2, sc % 2
            ps = psP.tile([P, TN], F32, tag="pp", name="psV")
            for kc in range(NCH):
                nc.tensor.matmul(
                    ps, state["encT"][:, kc, sc * P:(sc + 1) * P],
                    state["wv"][:, kc, nn * TN:(nn + 1) * TN],
                    start=(kc == 0), stop=(kc == NCH - 1))
            nh = TN // D
            dst = vS2[u][:, j, nn * nh * G:(nn + 1) * nh * G] \
                .rearrange("p (h g) -> p h g", g=G)[:, :, 0:D]
            srcp = ps.rearrange("p (h g) -> p h g", g=D)
            bsrc = state["bvb"][:, nn * TN:(nn + 1) * TN] \
                .rearrange("p (h g) -> p h g", g=D)
            nc.vector.tensor_add(dst, srcp, bsrc)

        def out_half1(tp, nn):
            # first-half contraction (y chunks 0..3 = heads 0..7) + bias,
            # staged in SBUF; second half finishes after the last head.
            ps = psP.tile([P, TN], F32, tag="pp", name="psO1")
            for kc in range(NCH // 2):
                nc.tensor.matmul(
                    ps, yT[:, kc, tp * P:(tp + 1) * P],
                    wo[:, kc, nn * TN:(nn + 1) * TN],
                    start=(kc == 0), stop=(kc == NCH // 2 - 1))
            nc.vector.tensor_add(
                state["o_part"][tp][:, nn * TN:(tn1 := (nn + 1)) * TN], ps,
                state["bob"][:, nn * TN:tn1 * TN])

        def attention_head(h, thunks, ti, thunk_slots=(2, 5)):
            c, ro = h // 2, (h % 2) * D
            ya = [psY.tile([D + 1, TN], F32, tag="ya", bufs=2,
                           name=f"ya{tn}") for tn in range(2)]
            px_tiles = []

            def attv(sc, start, stop):
                u, j = sc // 2, sc % 2
                for tn in range(2):
                    nc.tensor.matmul(
                        ya[tn], vS2[u][:, j, h * G:(h + 1) * G],
                        px_tiles[sc][:, tn * TN:(tn + 1) * TN],
                        start=start, stop=stop)

            for sc in range(NS):
                ps = psS.tile([P, TOK], F32, tag="ps", bufs=2, name="psS")
                for tn in range(2):
                    nc.tensor.matmul(
                        ps[:, tn * TN:(tn + 1) * TN],
                        kz[:, h, sc * P:(sc + 1) * P],
                        qT[:, c, tn * TN:(tn + 1) * TN],
                        start=True, stop=True)
                px = pPx.tile([P, TOK], BF16, tag="px", bufs=3, name="px")
                nc.scalar.activation(px, ps, AF.Exp, scale=float(SCALE))
                px_tiles.append(px)
                if dbg is not None and h == 0 and sc in (0, 7):
                    nc.sync.dma_start(out=dbg[f"d_px{sc}"], in_=px)
                if sc >= 1:
                    attv(sc - 1, start=(sc == 1), stop=False)
                if sc in thunk_slots and ti < len(thunks):
                    thunks[ti]()
                    ti += 1
            attv(NS - 1, start=False, stop=True)
            if dbg is not None and h == 0:
                for tn in range(2):
                    stg = pPx.tile([D + 1, TN], F32, tag=f"dbg{tn}",
                                   bufs=1, name="dbg")
                    nc.vector.tensor_copy(stg, ya[tn])
                    nc.sync.dma_start(out=dbg[f"d_ya{tn}"], in_=stg)

            for tn in range(2):
                # custom-DVE ops can't read PSUM on HW: stage the
                # denominator row to SBUF p0, then approx-reciprocal.
                den = pRc.tile([1, TN], F32, tag="den", bufs=2, name="den")
                nc.vector.tensor_copy(den, ya[tn][D:D + 1, :])
                rc = pRc.tile([1, TN], F32, tag="rc", bufs=2, name="rc")
                nc.vector.reciprocal_approx_fast(rc, den)
                bc = pBc.tile([D, TN], F32, tag="bc", bufs=2, name="bc")
                nc.gpsimd.partition_broadcast(bc, rc)
                nc.vector.tensor_mul(yT[ro:ro + D, c,
                                        tn * TN:(tn + 1) * TN],
                                     ya[tn][0:D, :], bc)
            return ti

        # ================= schedule =================
        with ExitStack() as S2:
            open_proj_scope(S2)
            # zero the off-head halves of kz once (idle Pool engine); the
            # K-proj copies below only ever write the in-head halves.
            kzv = kz.rearrange("p (x two) t -> p x two t", two=2)
            nc.gpsimd.memset(kzv[D:P, :, 0, :], 0.0)
            nc.gpsimd.memset(kzv[0:D, :, 1, :], 0.0)
            for c in range(NCH):
                for tn in range(2):
                    k_group(c, tn)
            for u in range(NS // 2):
                ones_dst = vS2[u].rearrange(
                    "p two (h g) -> p (two h) g", g=G)[:, :, D:D + 1]
                nc.vector.memset(ones_dst, 1.0)
            for sc in range(NS):
                v_group(sc, 0)
            kq_group(state["wq"], state["xT"], qT, state["bqc"], 0, 0)
            kq_group(state["wq"], state["xT"], qT, state["bqc"], 0, 1)

            thunks = []
            for c in range(1, NCH):
                thunks.append(
                    lambda c=c: kq_group(state["wq"], state["xT"], qT,
                                         state["bqc"], c, 0))
                thunks.append(
                    lambda c=c: kq_group(state["wq"], state["xT"], qT,
                                         state["bqc"], c, 1))
                if c <= 4:
                    sc0 = (c - 1) * 2
                    thunks.append(lambda sc=sc0: v_group(sc, 1))
                    thunks.append(lambda sc=sc0 + 1: v_group(sc, 1))

            if dbg is not None:
                nc.sync.dma_start(out=dbg["d_vS0"], in_=vS2[0])

            ti = 0
            for h in range(11):  # heads 0..10 consume all proj thunks
                ti = attention_head(h, thunks, ti)
            assert ti == len(thunks), (ti, len(thunks))

        # pTmp SBUF reclaimed; stage first-half out-proj panels there.
        with ExitStack() as S3:
            pO1 = S3.enter_context(tc.tile_pool(name="pO1", bufs=1))
            bob = pO1.tile([P, C], F32, tag="bob", name="bob")
            nc.gpsimd.partition_broadcast(bob, bo_row)
            state["bob"] = bob
            state["o_part"] = [pO1.tile([P, C], F32, tag=f"op{tp}",
                                        name=f"op{tp}")
                               for tp in range(TOK // P)]
            pO = S3.enter_context(tc.tile_pool(name="pO", bufs=2))

            thunks2 = [lambda tp=tp, nn=nn: out_half1(tp, nn)
                       for tp in range(TOK // P) for nn in range(2)]
            ti = 0
            for h in range(11, H):
                ti = attention_head(h, thunks2, ti, thunk_slots=(1, 3, 5))
            while ti < len(thunks2):
                thunks2[ti]()
                ti += 1

            if dbg is not None:
                nc.sync.dma_start(out=dbg["d_kT"], in_=kz)
                nc.sync.dma_start(out=dbg["d_qT"], in_=qT)
                nc.sync.dma_start(out=dbg["d_yT"], in_=yT)

            # ---- second-half contraction + staged first half -> out
            for tp in range(TOK // P):
                o_sb = pO.tile([P, C], F32, tag="o", name="o_sb")
                for nn in range(C // TN):
                    ps = psP.tile([P, TN], F32, tag="pp", name="psO2")
                    for kc in range(NCH // 2, NCH):
                        nc.tensor.matmul(
                            ps, yT[:, kc, tp * P:(tp + 1) * P],
                            wo[:, kc, nn * TN:(nn + 1) * TN],
                            start=(kc == NCH // 2), stop=(kc == NCH - 1))
                    nc.vector.tensor_add(
                        o_sb[:, nn * TN:(nn + 1) * TN], ps,
                        state["o_part"][tp][:, nn * TN:(nn + 1) * TN])
                q = nc.sync if tp % 2 == 0 else nc.scalar
                q.dma_start(out=out[tp * P:(tp + 1) * P, :], in_=o_sb)


def make_in_maps(inputs):
    """Full fp32 inputs -> per-core input maps (host-side shard + layout)."""
    import ml_dtypes

    bf16 = ml_dtypes.bfloat16
    x = np.asarray(inputs["x"], dtype=np.float32)
    enc = np.asarray(inputs["enc_x"], dtype=np.float32)
    half = x.shape[1] // 2

    def chunked_T(a2d):
        # [rows, C] fp32 -> [128, NCH, rows] bf16 with [p, c, r] = a[r, c*128+p]
        t = np.ascontiguousarray(
            a2d.T.reshape(NCH, P, a2d.shape[0]).transpose(1, 0, 2))
        return t.astype(bf16)

    weights = {}
    for name, key in (("wq", "Wq"), ("wk", "Wk"), ("wv", "Wv"), ("wo", "Wo")):
        W = np.asarray(inputs[key], dtype=np.float32)
        weights[name] = np.ascontiguousarray(
            W.reshape(NCH, P, C).transpose(1, 0, 2)).astype(bf16)
    weights["bqc"] = np.ascontiguousarray(
        np.asarray(inputs["bq"], np.float32).reshape(NCH, P).T)
    weights["bkc"] = np.ascontiguousarray(
        np.asarray(inputs["bk"], np.float32).reshape(NCH, P).T)
    weights["bv"] = np.asarray(inputs["bv"], np.float32)
    weights["bo"] = np.asarray(inputs["bo"], np.float32)

    encT = [chunked_T(enc[b]) for b in range(x.shape[0])]
    maps = []
    for core in range(N_CORES):
        b, th = core // 2, core % 2
        m = {"xT": chunked_T(x[b, th * half:(th + 1) * half, :]),
             "encT": encT[b]}
        m.update(weights)
        maps.append(m)
    return maps


_CACHED = None


def _get_program():
    global _CACHED
    if _CACHED is None:
        _CACHED = build_program()
    return _CACHED


def kernel(**inputs):
    x = np.asarray(inputs["x"], dtype=np.float32)
    B, T, Cx = x.shape
    assert (B, T, Cx) == (B_FULL, T_FULL, C), (B, T, Cx)
    half = T // 2

    nc = _get_program()
    in_maps = make_in_maps(inputs)

    from concourse.bass_utils import run_bass_kernel_spmd
    res = None
    last_err = None
    for _attempt in range(3):
        try:
            res = run_bass_kernel_spmd(nc, in_maps,
                                       core_ids=list(range(N_CORES)))
            break
        except Exception as e:  # transient NRT/axon failures: retry
            last_err = e
    if res is None:
        raise last_err

    outp = np.empty((B, T, C), dtype=np.float32)
    for core in range(N_CORES):
        b, th = core // 2, core % 2
        outp[b, th * half:(th + 1) * half, :] = res.results[core]["out"]
    return outp


if __name__ == "__main__":
    prog = build_program()
    n_inst = sum(len(blk.instructions) for fn in prog.m.functions
                 for blk in fn.blocks)
    print("built OK; instructions:", n_inst)


# revision 19
# speedup vs baseline: 1.0502x; 1.0502x over previous
"""Cross-attention Trainium2 kernel (8 NeuronCores, SPMD), v5.

Reference computation (per full batch):
  q = x @ Wq + bq;  k = enc @ Wk + bk;  v = enc @ Wv + bv
  att = softmax((q k^T) / sqrt(D));  y = (att v) @ Wo + bo

Sharding: B(=4) x T-half(=2) -> 8 cores. Each core handles one batch
element and half of the 2048 query tokens, all 16 heads, and writes
out[b, t_half] directly.

Design (vs the 415us f32r v0 baseline; measured ~295-308us, paired-min
~251us):
  * All matmul operands bf16 (1 cycle/row PE rate, half the LDWEIGHTS
    bytes of f32r, rel err ~3e-3 vs the 2e-2 budget).
  * Inputs arrive host-side pre-transposed and pre-cast:
      xT/encT  [128, 8, 1024] bf16   ([p, c, t] = x[t, c*128+p])
      wq/wk/wv/wo [128, 8, 1024] bf16 ([p, kc, n] = W[kc*128+p, n])
    eliminating all on-device PE transposes and their DVE copies.
  * kz [128, 16, 1024]: zero-padded per-head K so every matmul in the
    program runs the same (128,128) PE tile config -- mixing K=64
    scores with K=128 matmuls forces PE tile-config switches that
    expose every LDWEIGHTS (~40 ns/matmul cadence penalty). The pads
    are zeroed once by the idle Pool engine; K-proj writes only the
    in-head halves, so the zeros persist.
  * v is built directly in the att@v lhsT layout vS2[u] [128, 2, 16*65]
    (65-column groups per head: [v_h | ones]); the ones column makes
    psum row 64 of ya the softmax denominator for free, and the
    attention loop needs no per-head lhsT staging at all.
  * Softmax: exp on ACT (psum -> bf16, fused 1/sqrt(D) scale);
    normalization = DVE copy of the denominator row to SBUF (custom-DVE
    ops silently misread PSUM on HW), reciprocal_approx_fast, gpsimd
    partition_broadcast (source must start at partition 0 -- BIR
    verifier rejects other base partitions), one DVE multiply into yT.
  * Software-pipelined schedule: upfront K proj + V(heads 0-7 half) +
    Q(chunk 0) while chunked DMAs land in need-order across both HWDGE
    queues (sync: encT, wv-hi, xT; scalar: wk, wv-lo, wq, wo); the
    remaining Q/V projection groups and the first-half out-projection
    (y chunks 0-3, ready after head 7) are interleaved as filler work
    into the exp-paced attention heads; only the second-half out
    contraction remains as serial tail.
  * PSUM: scores 2x[128,1024] (4 banks) + ya 2x[65,512] (2) + shared
    projection/out psum 2x[128,512] (2) = exactly 8 banks.

Engine budget per core: PE ~245us busy (the bottleneck: 1024 matmuls
at ~220-240ns cadence incl LDWEIGHTS), ACT ~155us (128 exps of
[128,1024] at ~1.11us), DVE ~130us, Pool ~50us.
"""

import sys

sys.path.insert(0, "/opt/trn_rl_repo")

import numpy as np

import concourse.bass as bass  # noqa: E402,F401
import concourse.tile as tile  # noqa: E402
from concourse import bacc, mybir  # noqa: E402

F32 = mybir.dt.float32
BF16 = mybir.dt.bfloat16
AF = mybir.ActivationFunctionType

P = 128          # partitions
TOK = 1024       # query tokens per core
T2 = 1024        # kv sequence length
C = 1024         # embed dim
H = 16           # heads
D = 64           # head dim
NCH = C // P     # 8 channel chunks
NS = T2 // P     # 8 kv-position chunks
TN = 512         # matmul moving-dim tile
G = D + 1        # v-group stride in vS2 (64 v cols + ones col)
SCALE = 1.0 / np.sqrt(D)

N_CORES = 8
B_FULL, T_FULL = 4, 2048


def build_program(loop_iters=None, debug=False):
    """loop_iters: if set, wrap the body in a For_i hardware loop (timing)."""
    nc = bacc.Bacc("TRN2", target_bir_lowering=False, debug=False,
                   num_devices=N_CORES)

    aps = {}
    for name in ("xT", "encT", "wq", "wk", "wv", "wo"):
        aps[name] = nc.dram_tensor(name, [P, NCH, 1024], BF16,
                                   kind="ExternalInput").ap()
    for name in ("bqc", "bkc"):
        aps[name] = nc.dram_tensor(name, [P, NCH], F32,
                                   kind="ExternalInput").ap()
    for name in ("bv", "bo"):
        aps[name] = nc.dram_tensor(name, [C], F32, kind="ExternalInput").ap()
    out = nc.dram_tensor("out", [TOK, C], F32, kind="ExternalOutput").ap()

    dbg = None
    if debug:
        dbg = {}
        for name, shape, dt in (
                ("d_kT", [P, H, T2], BF16), ("d_qT", [P, NCH, TOK], BF16),
                ("d_yT", [P, NCH, TOK], BF16),
                ("d_vS0", [P, 2, H * G], BF16),
                ("d_px0", [P, TOK], BF16), ("d_px7", [P, TOK], BF16),
                ("d_ya0", [D + 1, TN], F32), ("d_ya1", [D + 1, TN], F32)):
            dbg[name] = nc.dram_tensor(name, shape, dt,
                                       kind="ExternalOutput").ap()

    with tile.TileContext(nc) as tc:
        if loop_iters is not None:
            with tc.For_i(0, loop_iters, 1):
                _emit(nc, tc, aps, out)
        else:
            _emit(nc, tc, aps, out, dbg)

    nc.compile()
    return nc


def _row(ap):
    return ap.rearrange("(a c) -> a c", a=1)


def _emit(nc, tc, aps, out, dbg=None):
    from contextlib import ExitStack

    with ExitStack() as S:
        pIn = S.enter_context(tc.tile_pool(name="pIn", bufs=1))

        # ---- persistent tiles
        wo = pIn.tile([P, NCH, C], BF16, tag="wo", name="wo")
        bo_row = pIn.tile([1, C], F32, tag="bo_row", name="bo_row")
        kz = pIn.tile([P, H, T2], BF16, tag="kz", name="kz")
        qT = pIn.tile([P, NCH, TOK], BF16, tag="qT", name="qT")
        yT = pIn.tile([P, NCH, TOK], BF16, tag="yT", name="yT")
        vS2 = [pIn.tile([P, 2, H * G], BF16, tag=f"vS2_{u}", name=f"vS2_{u}")
               for u in range(NS // 2)]

        psP = S.enter_context(tc.tile_pool(name="psP", bufs=2, space="PSUM"))
        psS = S.enter_context(tc.tile_pool(name="psS", bufs=2, space="PSUM"))
        psY = S.enter_context(tc.tile_pool(name="psY", bufs=2, space="PSUM"))
        pPx = S.enter_context(tc.tile_pool(name="pPx", bufs=3))
        pRc = S.enter_context(tc.tile_pool(name="pRc", bufs=2))
        pBc = S.enter_context(tc.tile_pool(name="pBc", bufs=2))

        state = {}

        def open_proj_scope(S2):
            # DMA priority: K-proj inputs first on both HWDGE queues, then
            # wv split across both, then Q-proj inputs, wo last.
            pTmp = S2.enter_context(tc.tile_pool(name="pTmp", bufs=1))
            bkc = pTmp.tile([P, NCH], F32, tag="bkc", name="bkc")
            nc.sync.dma_start(out=bkc, in_=aps["bkc"])
            bqc = pTmp.tile([P, NCH], F32, tag="bqc", name="bqc")
            nc.sync.dma_start(out=bqc, in_=aps["bqc"])
            bv_row = pTmp.tile([1, C], F32, tag="bv_row", name="bv_row")
            nc.sync.dma_start(out=bv_row, in_=_row(aps["bv"]))
            nc.sync.dma_start(out=bo_row, in_=_row(aps["bo"]))

            encT = pTmp.tile([P, NCH, T2], BF16, tag="encT", name="encT")
            wk = pTmp.tile([P, NCH, C], BF16, tag="wk", name="wk")
            wv = pTmp.tile([P, NCH, C], BF16, tag="wv", name="wv")
            wq = pTmp.tile([P, NCH, C], BF16, tag="wq", name="wq")
            xT = pTmp.tile([P, NCH, TOK], BF16, tag="xT", name="xT")
            for kc in range(2):
                nc.scalar.dma_start(out=encT[:, kc, :],
                                    in_=aps["encT"][:, kc, :])
            for kc in range(NCH):
                if kc >= 2:
                    nc.sync.dma_start(out=encT[:, kc, :],
                                      in_=aps["encT"][:, kc, :])
                nc.scalar.dma_start(out=wk[:, kc, :], in_=aps["wk"][:, kc, :])
            for kc in range(NCH):
                q = nc.sync if kc >= 4 else nc.scalar
                q.dma_start(out=wv[:, kc, :], in_=aps["wv"][:, kc, :])
            for kc in range(NCH):
                nc.sync.dma_start(out=xT[:, kc, :], in_=aps["xT"][:, kc, :])
            for kc in range(NCH):
                nc.scalar.dma_start(out=wq[:, kc, :], in_=aps["wq"][:, kc, :])
            nc.scalar.dma_start(out=wo, in_=aps["wo"])

            bvb = pTmp.tile([P, C], F32, tag="bvb", name="bvb")
            nc.gpsimd.partition_broadcast(bvb, bv_row)
            state.update(encT=encT, wk=wk, wv=wv, wq=wq, xT=xT,
                         bkc=bkc, bqc=bqc, bvb=bvb)

        def k_group(c, tn):
            ps = psP.tile([P, TN], F32, tag="pp", name="psK")
            for kc in range(NCH):
                nc.tensor.matmul(
                    ps, state["wk"][:, kc, c * P:(c + 1) * P],
                    state["encT"][:, kc, tn * TN:(tn + 1) * TN],
                    start=(kc == 0), stop=(kc == NCH - 1))
            bkc = state["bkc"]
            ts = slice(tn * TN, (tn + 1) * TN)
            nc.vector.tensor_scalar_add(kz[0:D, 2 * c, ts], ps[0:D, :],
                                        bkc[0:D, c:c + 1])
            nc.vector.tensor_scalar_add(kz[D:P, 2 * c + 1, ts], ps[D:P, :],
                                        bkc[D:P, c:c + 1])

        def kq_group(w, src, dst, bcol, c, tn):
            ps = psP.tile([P, TN], F32, tag="pp", name="psP")
            for kc in range(NCH):
                nc.tensor.matmul(
                    ps, w[:, kc, c * P:(c + 1) * P],
                    src[:, kc, tn * TN:(tn + 1) * TN],
                    start=(kc == 0), stop=(kc == NCH - 1))
            nc.vector.tensor_scalar_add(dst[:, c, tn * TN:(tn + 1) * TN], ps,
                                        bcol[:, c:c + 1])

        def v_group(sc, nn):
            u, j = sc // 2, sc % 2
            ps = psP.tile([P, TN], F32, tag="pp", name="psV")
            for kc in range(NCH):
                nc.tensor.matmul(
                    ps, state["encT"][:, kc, sc * P:(sc + 1) * P],
                    state["wv"][:, kc, nn * TN:(nn + 1) * TN],
                    start=(kc == 0), stop=(kc == NCH - 1))
            nh = TN // D
            dst = vS2[u][:, j, nn * nh * G:(nn + 1) * nh * G] \
                .rearrange("p (h g) -> p h g", g=G)[:, :, 0:D]
            srcp = ps.rearrange("p (h g) -> p h g", g=D)
            bsrc = state["bvb"][:, nn * TN:(nn + 1) * TN] \
                .rearrange("p (h g) -> p h g", g=D)
            nc.vector.tensor_add(dst, srcp, bsrc)

        def out_half1(tp, nn):
            # first-half contraction (y chunks 0..3 = heads 0..7) + bias,
            # staged in SBUF; second half finishes after the last head.
            ps = psP.tile([P, TN], F32, tag="pp", name="psO1")
            for kc in range(NCH // 2):
                nc.tensor.matmul(
                    ps, yT[:, kc, tp * P:(tp + 1) * P],
                    wo[:, kc, nn * TN:(nn + 1) * TN],
                    start=(kc == 0), stop=(kc == NCH // 2 - 1))
            nc.vector.tensor_add(
                state["o_part"][tp][:, nn * TN:(tn1 := (nn + 1)) * TN], ps,
                state["bob"][:, nn * TN:tn1 * TN])

        def attention_head(h, thunks, ti, thunk_slots=(2, 5)):
            c, ro = h // 2, (h % 2) * D
            ya = [psY.tile([D + 1, TN], F32, tag="ya", bufs=2,
                           name=f"ya{tn}") for tn in range(2)]
            px_tiles = []

            def attv(sc, start, stop):
                u, j = sc // 2, sc % 2
                for tn in range(2):
                    nc.tensor.matmul(
                        ya[tn], vS2[u][:, j, h * G:(h + 1) * G],
                        px_tiles[sc][:, tn * TN:(tn + 1) * TN],
                        start=start, stop=stop)

            for sc in range(NS):
                if sc >= 2:
                    attv(sc - 2, start=(sc == 2), stop=False)
                ps = psS.tile([P, TOK], F32, tag="ps", bufs=2, name="psS")
                for tn in range(2):
                    nc.tensor.matmul(
                        ps[:, tn * TN:(tn + 1) * TN],
                        kz[:, h, sc * P:(sc + 1) * P],
                        qT[:, c, tn * TN:(tn + 1) * TN],
                        start=True, stop=True)
                px = pPx.tile([P, TOK], BF16, tag="px", bufs=3, name="px")
                nc.scalar.activation(px, ps, AF.Exp, scale=float(SCALE))
                px_tiles.append(px)
                if dbg is not None and h == 0 and sc in (0, 7):
                    nc.sync.dma_start(out=dbg[f"d_px{sc}"], in_=px)
                if sc in thunk_slots and ti < len(thunks):
                    thunks[ti]()
                    ti += 1
            attv(NS - 2, start=False, stop=False)
            attv(NS - 1, start=False, stop=True)
            if dbg is not None and h == 0:
                for tn in range(2):
                    stg = pPx.tile([D + 1, TN], F32, tag=f"dbg{tn}",
                                   bufs=1, name="dbg")
                    nc.vector.tensor_copy(stg, ya[tn])
                    nc.sync.dma_start(out=dbg[f"d_ya{tn}"], in_=stg)

            for tn in range(2):
                # custom-DVE ops can't read PSUM on HW: stage the
                # denominator row to SBUF p0, then approx-reciprocal.
                den = pRc.tile([1, TN], F32, tag="den", bufs=2, name="den")
                nc.vector.tensor_copy(den, ya[tn][D:D + 1, :])
                rc = pRc.tile([1, TN], F32, tag="rc", bufs=2, name="rc")
                nc.vector.reciprocal_approx_fast(rc, den)
                bc = pBc.tile([D, TN], F32, tag="bc", bufs=2, name="bc")
                nc.gpsimd.partition_broadcast(bc, rc)
                nc.vector.tensor_mul(yT[ro:ro + D, c,
                                        tn * TN:(tn + 1) * TN],
                                     ya[tn][0:D, :], bc)
            return ti

        # ================= schedule =================
        with ExitStack() as S2:
            open_proj_scope(S2)
            # zero the off-head halves of kz once (idle Pool engine); the
            # K-proj copies below only ever write the in-head halves.
            kzv = kz.rearrange("p (x two) t -> p x two t", two=2)
            nc.gpsimd.memset(kzv[D:P, :, 0, :], 0.0)
            nc.gpsimd.memset(kzv[0:D, :, 1, :], 0.0)
            for c in range(NCH):
                for tn in range(2):
                    k_group(c, tn)
            for u in range(NS // 2):
                ones_dst = vS2[u].rearrange(
                    "p two (h g) -> p (two h) g", g=G)[:, :, D:D + 1]
                nc.vector.memset(ones_dst, 1.0)
            for sc in range(NS):
                v_group(sc, 0)
            kq_group(state["wq"], state["xT"], qT, state["bqc"], 0, 0)
            kq_group(state["wq"], state["xT"], qT, state["bqc"], 0, 1)

            thunks = []
            for c in range(1, NCH):
                thunks.append(
                    lambda c=c: kq_group(state["wq"], state["xT"], qT,
                                         state["bqc"], c, 0))
                thunks.append(
                    lambda c=c: kq_group(state["wq"], state["xT"], qT,
                                         state["bqc"], c, 1))
                if c <= 4:
                    sc0 = (c - 1) * 2
                    thunks.append(lambda sc=sc0: v_group(sc, 1))
                    thunks.append(lambda sc=sc0 + 1: v_group(sc, 1))

            if dbg is not None:
                nc.sync.dma_start(out=dbg["d_vS0"], in_=vS2[0])

            ti = 0
            for h in range(11):  # heads 0..10 consume all proj thunks
                ti = attention_head(h, thunks, ti)
            assert ti == len(thunks), (ti, len(thunks))

        # pTmp SBUF reclaimed; stage first-half out-proj panels there.
        with ExitStack() as S3:
            pO1 = S3.enter_context(tc.tile_pool(name="pO1", bufs=1))
            bob = pO1.tile([P, C], F32, tag="bob", name="bob")
            nc.gpsimd.partition_broadcast(bob, bo_row)
            state["bob"] = bob
            state["o_part"] = [pO1.tile([P, C], F32, tag=f"op{tp}",
                                        name=f"op{tp}")
                               for tp in range(TOK // P)]
            pO = S3.enter_context(tc.tile_pool(name="pO", bufs=2))

            thunks2 = [lambda tp=tp, nn=nn: out_half1(tp, nn)
                       for tp in range(TOK // P) for nn in range(2)]
            ti = 0
            for h in range(11, H):
                ti = attention_head(h, thunks2, ti, thunk_slots=(1, 3, 5))
            while ti < len(thunks2):
                thunks2[ti]()
                ti += 1

            if dbg is not None:
                nc.sync.dma_start(out=dbg["d_kT"], in_=kz)
                nc.sync.dma_start(out=dbg["d_qT"], in_=qT)
                nc.sync.dma_start(out=dbg["d_yT"], in_=yT)

            # ---- second-half contraction + staged first half -> out
            for tp in range(TOK // P):
                o_sb = pO.tile([P, C], F32, tag="o", name="o_sb")
                for nn in range(C // TN):
                    ps = psP.tile([P, TN], F32, tag="pp", name="psO2")
                    for kc in range(NCH // 2, NCH):
                        nc.tensor.matmul(
                            ps, yT[:, kc, tp * P:(tp + 1) * P],
                            wo[:, kc, nn * TN:(nn + 1) * TN],
                            start=(kc == NCH // 2), stop=(kc == NCH - 1))
                    nc.vector.tensor_add(
                        o_sb[:, nn * TN:(nn + 1) * TN], ps,
                        state["o_part"][tp][:, nn * TN:(nn + 1) * TN])
                q = nc.sync if tp % 2 == 0 else nc.scalar
                q.dma_start(out=out[tp * P:(tp + 1) * P, :], in_=o_sb)


def make_in_maps(inputs):
    """Full fp32 inputs -> per-core input maps (host-side shard + layout)."""
    import ml_dtypes

    bf16 = ml_dtypes.bfloat16
    x = np.asarray(inputs["x"], dtype=np.float32)
    enc = np.asarray(inputs["enc_x"], dtype=np.float32)
    half = x.shape[1] // 2

    def chunked_T(a2d):
        # [rows, C] fp32 -> [128, NCH, rows] bf16 with [p, c, r] = a[r, c*128+p]
        t = np.ascontiguousarray(
            a2d.T.reshape(NCH, P, a2d.shape[0]).transpose(1, 0, 2))
        return t.astype(bf16)

    weights = {}
    for name, key in (("wq", "Wq"), ("wk", "Wk"), ("wv", "Wv"), ("wo", "Wo")):
        W = np.asarray(inputs[key], dtype=np.float32)
        weights[name] = np.ascontiguousarray(
            W.reshape(NCH, P, C).transpose(1, 0, 2)).astype(bf16)
    weights["bqc"] = np.ascontiguousarray(
        np.asarray(inputs["bq"], np.float32).reshape(NCH, P).T)
    weights["bkc"] = np.ascontiguousarray(
        np.asarray(inputs["bk"], np.float32).reshape(NCH, P).T)
    weights["bv"] = np.asarray(inputs["bv"], np.float32)
    weights["bo"] = np.asarray(inputs["bo"], np.float32)

    encT = [chunked_T(enc[b]) for b in range(x.shape[0])]
    maps = []
    for core in range(N_CORES):
        b, th = core // 2, core % 2
        m = {"xT": chunked_T(x[b, th * half:(th + 1) * half, :]),
             "encT": encT[b]}
        m.update(weights)
        maps.append(m)
    return maps


_CACHED = None


def _get_program():
    global _CACHED
    if _CACHED is None:
        _CACHED = build_program()
    return _CACHED


def kernel(**inputs):
    x = np.asarray(inputs["x"], dtype=np.float32)
    B, T, Cx = x.shape
    assert (B, T, Cx) == (B_FULL, T_FULL, C), (B, T, Cx)
    half = T // 2

    nc = _get_program()
    in_maps = make_in_maps(inputs)

    from concourse.bass_utils import run_bass_kernel_spmd
    res = None
    last_err = None
    for _attempt in range(3):
        try:
            res = run_bass_kernel_spmd(nc, in_maps,
                                       core_ids=list(range(N_CORES)))
            break
        except Exception as e:  # transient NRT/axon failures: retry
            last_err = e
    if res is None:
        raise last_err

    outp = np.empty((B, T, C), dtype=np.float32)
    for core in range(N_CORES):
        b, th = core // 2, core % 2
        outp[b, th * half:(th + 1) * half, :] = res.results[core]["out"]
    return outp


if __name__ == "__main__":
    prog = build_program()
    n_inst = sum(len(blk.instructions) for fn in prog.m.functions
                 for blk in fn.blocks)
    print("built OK; instructions:", n_inst)


# revision 23
# speedup vs baseline: 1.0666x; 1.0156x over previous
"""Cross-attention Trainium2 kernel (8 NeuronCores, SPMD), v5.

Reference computation (per full batch):
  q = x @ Wq + bq;  k = enc @ Wk + bk;  v = enc @ Wv + bv
  att = softmax((q k^T) / sqrt(D));  y = (att v) @ Wo + bo

Sharding: B(=4) x T-half(=2) -> 8 cores. Each core handles one batch
element and half of the 2048 query tokens, all 16 heads, and writes
out[b, t_half] directly.

Design (vs the 415us f32r v0 baseline; measured ~295-308us, paired-min
~251us):
  * All matmul operands bf16 (1 cycle/row PE rate, half the LDWEIGHTS
    bytes of f32r, rel err ~3e-3 vs the 2e-2 budget).
  * Inputs arrive host-side pre-transposed and pre-cast:
      xT/encT  [128, 8, 1024] bf16   ([p, c, t] = x[t, c*128+p])
      wq/wk/wv/wo [128, 8, 1024] bf16 ([p, kc, n] = W[kc*128+p, n])
    eliminating all on-device PE transposes and their DVE copies.
  * kz [128, 16, 1024]: zero-padded per-head K so every matmul in the
    program runs the same (128,128) PE tile config -- mixing K=64
    scores with K=128 matmuls forces PE tile-config switches that
    expose every LDWEIGHTS (~40 ns/matmul cadence penalty). The pads
    are zeroed once by the idle Pool engine; K-proj writes only the
    in-head halves, so the zeros persist.
  * v is built directly in the att@v lhsT layout vS2[u] [128, 2, 16*65]
    (65-column groups per head: [v_h | ones]); the ones column makes
    psum row 64 of ya the softmax denominator for free, and the
    attention loop needs no per-head lhsT staging at all.
  * Softmax: exp on ACT (psum -> bf16, fused 1/sqrt(D) scale);
    normalization = DVE copy of the denominator row to SBUF (custom-DVE
    ops silently misread PSUM on HW), reciprocal_approx_fast, gpsimd
    partition_broadcast (source must start at partition 0 -- BIR
    verifier rejects other base partitions), one DVE multiply into yT.
  * Software-pipelined schedule: upfront K proj + V(heads 0-7 half) +
    Q(chunk 0) while chunked DMAs land in need-order across both HWDGE
    queues (sync: encT, wv-hi, xT; scalar: wk, wv-lo, wq, wo); the
    remaining Q/V projection groups and the first-half out-projection
    (y chunks 0-3, ready after head 7) are interleaved as filler work
    into the exp-paced attention heads; only the second-half out
    contraction remains as serial tail.
  * PSUM: scores 2x[128,1024] (4 banks) + ya 2x[65,512] (2) + shared
    projection/out psum 2x[128,512] (2) = exactly 8 banks.

Engine budget per core: PE ~245us busy (the bottleneck: 1024 matmuls
at ~220-240ns cadence incl LDWEIGHTS), ACT ~155us (128 exps of
[128,1024] at ~1.11us), DVE ~130us, Pool ~50us.
"""

import sys

sys.path.insert(0, "/opt/trn_rl_repo")

import numpy as np

import concourse.bass as bass  # noqa: E402,F401
import concourse.tile as tile  # noqa: E402
from concourse import bacc, mybir  # noqa: E402

F32 = mybir.dt.float32
BF16 = mybir.dt.bfloat16
AF = mybir.ActivationFunctionType

P = 128          # partitions
TOK = 1024       # query tokens per core
T2 = 1024        # kv sequence length
C = 1024         # embed dim
H = 16           # heads
D = 64           # head dim
NCH = C // P     # 8 channel chunks
NS = T2 // P     # 8 kv-position chunks
TN = 512         # matmul moving-dim tile
G = D + 1        # v-group stride in vS2 (64 v cols + ones col)
SCALE = 1.0 / np.sqrt(D)

N_CORES = 8
B_FULL, T_FULL = 4, 2048


def build_program(loop_iters=None, debug=False):
    """loop_iters: if set, wrap the body in a For_i hardware loop (timing)."""
    nc = bacc.Bacc("TRN2", target_bir_lowering=False, debug=False,
                   num_devices=N_CORES)

    aps = {}
    for name in ("xT", "encT", "wq", "wk", "wv", "wo"):
        aps[name] = nc.dram_tensor(name, [P, NCH, 1024], BF16,
                                   kind="ExternalInput").ap()
    for name in ("bqc", "bkc"):
        aps[name] = nc.dram_tensor(name, [P, NCH], F32,
                                   kind="ExternalInput").ap()
    for name in ("bv", "bo"):
        aps[name] = nc.dram_tensor(name, [C], F32, kind="ExternalInput").ap()
    out = nc.dram_tensor("out", [TOK, C], F32, kind="ExternalOutput").ap()

    dbg = None
    if debug:
        dbg = {}
        for name, shape, dt in (
                ("d_kT", [P, H, T2], BF16), ("d_qT", [P, NCH, TOK], BF16),
                ("d_yT", [P, NCH, TOK], BF16),
                ("d_vS0", [P, 2, H * G], BF16),
                ("d_px0", [P, TOK], BF16), ("d_px7", [P, TOK], BF16),
                ("d_ya0", [D + 1, TN], F32), ("d_ya1", [D + 1, TN], F32)):
            dbg[name] = nc.dram_tensor(name, shape, dt,
                                       kind="ExternalOutput").ap()

    with tile.TileContext(nc) as tc:
        if loop_iters is not None:
            with tc.For_i(0, loop_iters, 1):
                _emit(nc, tc, aps, out)
        else:
            _emit(nc, tc, aps, out, dbg)

    nc.compile()
    return nc


def _row(ap):
    return ap.rearrange("(a c) -> a c", a=1)


def _emit(nc, tc, aps, out, dbg=None):
    from contextlib import ExitStack

    with ExitStack() as S:
        pIn = S.enter_context(tc.tile_pool(name="pIn", bufs=1))

        # ---- persistent tiles
        wo = pIn.tile([P, NCH, C], BF16, tag="wo", name="wo")
        bo_row = pIn.tile([1, C], F32, tag="bo_row", name="bo_row")
        kz = pIn.tile([P, H, T2], BF16, tag="kz", name="kz")
        qT = pIn.tile([P, NCH, TOK], BF16, tag="qT", name="qT")
        yT = pIn.tile([P, NCH, TOK], BF16, tag="yT", name="yT")
        vS2 = [pIn.tile([P, 2, H * G], BF16, tag=f"vS2_{u}", name=f"vS2_{u}")
               for u in range(NS // 2)]

        psP = S.enter_context(tc.tile_pool(name="psP", bufs=2, space="PSUM"))
        psS = S.enter_context(tc.tile_pool(name="psS", bufs=2, space="PSUM"))
        psY = S.enter_context(tc.tile_pool(name="psY", bufs=2, space="PSUM"))
        pPx = S.enter_context(tc.tile_pool(name="pPx", bufs=3))
        pRc = S.enter_context(tc.tile_pool(name="pRc", bufs=2))
        pBc = S.enter_context(tc.tile_pool(name="pBc", bufs=2))

        state = {}

        def open_proj_scope(S2):
            # DMA priority: K-proj inputs first on both HWDGE queues, then
            # wv split across both, then Q-proj inputs, wo last.
            pTmp = S2.enter_context(tc.tile_pool(name="pTmp", bufs=1))
            bkc = pTmp.tile([P, NCH], F32, tag="bkc", name="bkc")
            nc.sync.dma_start(out=bkc, in_=aps["bkc"])
            bqc = pTmp.tile([P, NCH], F32, tag="bqc", name="bqc")
            nc.sync.dma_start(out=bqc, in_=aps["bqc"])
            bv_row = pTmp.tile([1, C], F32, tag="bv_row", name="bv_row")
            nc.sync.dma_start(out=bv_row, in_=_row(aps["bv"]))
            nc.sync.dma_start(out=bo_row, in_=_row(aps["bo"]))

            encT = pTmp.tile([P, NCH, T2], BF16, tag="encT", name="encT")
            wk = pTmp.tile([P, NCH, C], BF16, tag="wk", name="wk")
            wv = pTmp.tile([P, NCH, C], BF16, tag="wv", name="wv")
            wq = pTmp.tile([P, NCH, C], BF16, tag="wq", name="wq")
            xT = pTmp.tile([P, NCH, TOK], BF16, tag="xT", name="xT")
            for kc in range(2):
                nc.scalar.dma_start(out=encT[:, kc, :],
                                    in_=aps["encT"][:, kc, :])
            for kc in range(NCH):
                if kc >= 2:
                    nc.sync.dma_start(out=encT[:, kc, :],
                                      in_=aps["encT"][:, kc, :])
                nc.scalar.dma_start(out=wk[:, kc, :], in_=aps["wk"][:, kc, :])
            for kc in range(NCH):
                q = nc.sync if kc >= 4 else nc.scalar
                q.dma_start(out=wv[:, kc, :], in_=aps["wv"][:, kc, :])
            for kc in range(NCH):
                nc.sync.dma_start(out=xT[:, kc, :], in_=aps["xT"][:, kc, :])
            for kc in range(NCH):
                nc.scalar.dma_start(out=wq[:, kc, :], in_=aps["wq"][:, kc, :])
            nc.scalar.dma_start(out=wo, in_=aps["wo"])

            bvb = pTmp.tile([P, C], F32, tag="bvb", name="bvb")
            nc.gpsimd.partition_broadcast(bvb, bv_row)
            state.update(encT=encT, wk=wk, wv=wv, wq=wq, xT=xT,
                         bkc=bkc, bqc=bqc, bvb=bvb)

        def k_group(c, tn):
            ps = psP.tile([P, TN], F32, tag="pp", name="psK")
            for kc in range(NCH):
                nc.tensor.matmul(
                    ps, state["wk"][:, kc, c * P:(c + 1) * P],
                    state["encT"][:, kc, tn * TN:(tn + 1) * TN],
                    start=(kc == 0), stop=(kc == NCH - 1))
            bkc = state["bkc"]
            ts = slice(tn * TN, (tn + 1) * TN)
            nc.vector.tensor_scalar_add(kz[0:D, 2 * c, ts], ps[0:D, :],
                                        bkc[0:D, c:c + 1])
            nc.vector.tensor_scalar_add(kz[D:P, 2 * c + 1, ts], ps[D:P, :],
                                        bkc[D:P, c:c + 1])

        def kq_group(w, src, dst, bcol, c, tn):
            ps = psP.tile([P, TN], F32, tag="pp", name="psP")
            for kc in range(NCH):
                nc.tensor.matmul(
                    ps, w[:, kc, c * P:(c + 1) * P],
                    src[:, kc, tn * TN:(tn + 1) * TN],
                    start=(kc == 0), stop=(kc == NCH - 1))
            nc.vector.tensor_scalar_add(dst[:, c, tn * TN:(tn + 1) * TN], ps,
                                        bcol[:, c:c + 1])

        def v_group(sc, nn):
            u, j = sc // 2, sc % 2
            ps = psP.tile([P, TN], F32, tag="pp", name="psV")
            for kc in range(NCH):
                nc.tensor.matmul(
                    ps, state["encT"][:, kc, sc * P:(sc + 1) * P],
                    state["wv"][:, kc, nn * TN:(nn + 1) * TN],
                    start=(kc == 0), stop=(kc == NCH - 1))
            nh = TN // D
            dst = vS2[u][:, j, nn * nh * G:(nn + 1) * nh * G] \
                .rearrange("p (h g) -> p h g", g=G)[:, :, 0:D]
            srcp = ps.rearrange("p (h g) -> p h g", g=D)
            bsrc = state["bvb"][:, nn * TN:(nn + 1) * TN] \
                .rearrange("p (h g) -> p h g", g=D)
            nc.vector.tensor_add(dst, srcp, bsrc)

        def out_half1(tp, nn):
            # first-half contraction (y chunks 0..3 = heads 0..7) + bias,
            # staged in SBUF; second half finishes after the last head.
            ps = psP.tile([P, TN], F32, tag="pp", name="psO1")
            for kc in range(NCH // 2):
                nc.tensor.matmul(
                    ps, yT[:, kc, tp * P:(tp + 1) * P],
                    wo[:, kc, nn * TN:(nn + 1) * TN],
                    start=(kc == 0), stop=(kc == NCH // 2 - 1))
            nc.vector.tensor_add(
                state["o_part"][tp][:, nn * TN:(tn1 := (nn + 1)) * TN], ps,
                state["bob"][:, nn * TN:tn1 * TN])

        def attention_head(h, thunks, ti, thunk_slots=(2, 5)):
            c, ro = h // 2, (h % 2) * D
            ya = [psY.tile([D + 1, TN], F32, tag="ya", bufs=2,
                           name=f"ya{tn}") for tn in range(2)]
            px_tiles = []

            def attv(sc, start, stop):
                u, j = sc // 2, sc % 2
                for tn in range(2):
                    nc.tensor.matmul(
                        ya[tn], vS2[u][:, j, h * G:(h + 1) * G],
                        px_tiles[sc][:, tn * TN:(tn + 1) * TN],
                        start=start, stop=stop)

            for sc in range(NS):
                if sc >= 2:
                    attv(sc - 2, start=(sc == 2), stop=False)
                ps = psS.tile([P, TOK], F32, tag="ps", bufs=2, name="psS")
                for tn in range(2):
                    nc.tensor.matmul(
                        ps[:, tn * TN:(tn + 1) * TN],
                        kz[:, h, sc * P:(sc + 1) * P],
                        qT[:, c, tn * TN:(tn + 1) * TN],
                        start=True, stop=True)
                px = pPx.tile([P, TOK], BF16, tag="px", bufs=3, name="px")
                nc.scalar.activation(px, ps, AF.Exp, scale=float(SCALE))
                px_tiles.append(px)
                if dbg is not None and h == 0 and sc in (0, 7):
                    nc.sync.dma_start(out=dbg[f"d_px{sc}"], in_=px)
                if sc in thunk_slots and ti < len(thunks):
                    thunks[ti]()
                    ti += 1
            attv(NS - 2, start=False, stop=False)
            attv(NS - 1, start=False, stop=True)
            if dbg is not None and h == 0:
                for tn in range(2):
                    stg = pPx.tile([D + 1, TN], F32, tag=f"dbg{tn}",
                                   bufs=1, name="dbg")
                    nc.vector.tensor_copy(stg, ya[tn])
                    nc.sync.dma_start(out=dbg[f"d_ya{tn}"], in_=stg)

            for tn in range(2):
                # custom-DVE ops can't read PSUM on HW: stage the
                # denominator row to SBUF p0, then approx-reciprocal.
                den = pRc.tile([1, TN], F32, tag="den", bufs=2, name="den")
                nc.vector.tensor_copy(den, ya[tn][D:D + 1, :])
                rc = pRc.tile([1, TN], F32, tag="rc", bufs=2, name="rc")
                nc.vector.reciprocal_approx_fast(rc, den)
                bc = pBc.tile([D, TN], F32, tag="bc", bufs=2, name="bc")
                nc.gpsimd.partition_broadcast(bc, rc)
                nc.vector.tensor_mul(yT[ro:ro + D, c,
                                        tn * TN:(tn + 1) * TN],
                                     ya[tn][0:D, :], bc)
            return ti

        # ================= schedule =================
        with ExitStack() as S2:
            open_proj_scope(S2)
            # zero the off-head halves of kz once (idle Pool engine); the
            # K-proj copies below only ever write the in-head halves.
            kzv = kz.rearrange("p (x two) t -> p x two t", two=2)
            nc.gpsimd.memset(kzv[D:P, :, 0, :], 0.0)
            nc.gpsimd.memset(kzv[0:D, :, 1, :], 0.0)
            for c in range(NCH):
                for tn in range(2):
                    k_group(c, tn)
            for u in range(NS // 2):
                ones_dst = vS2[u].rearrange(
                    "p two (h g) -> p (two h) g", g=G)[:, :, D:D + 1]
                nc.vector.memset(ones_dst, 1.0)
            for sc in range(NS):
                v_group(sc, 0)
            kq_group(state["wq"], state["xT"], qT, state["bqc"], 0, 0)
            kq_group(state["wq"], state["xT"], qT, state["bqc"], 0, 1)

            thunks = []
            for c in range(1, NCH):
                thunks.append(
                    lambda c=c: kq_group(state["wq"], state["xT"], qT,
                                         state["bqc"], c, 0))
                thunks.append(
                    lambda c=c: kq_group(state["wq"], state["xT"], qT,
                                         state["bqc"], c, 1))
                if c <= 4:
                    sc0 = (c - 1) * 2
                    thunks.append(lambda sc=sc0: v_group(sc, 1))
                    thunks.append(lambda sc=sc0 + 1: v_group(sc, 1))

            if dbg is not None:
                nc.sync.dma_start(out=dbg["d_vS0"], in_=vS2[0])

            ti = 0
            for h in range(11):  # heads 0..10 consume all proj thunks
                ti = attention_head(h, thunks, ti)
            assert ti == len(thunks), (ti, len(thunks))

        # pTmp SBUF reclaimed; stage first-half out-proj panels there.
        with ExitStack() as S3:
            pO1 = S3.enter_context(tc.tile_pool(name="pO1", bufs=1))
            bob = pO1.tile([P, C], F32, tag="bob", name="bob")
            nc.gpsimd.partition_broadcast(bob, bo_row)
            state["bob"] = bob
            state["o_part"] = [pO1.tile([P, C], F32, tag=f"op{tp}",
                                        name=f"op{tp}")
                               for tp in range(TOK // P)]
            pO = S3.enter_context(tc.tile_pool(name="pO", bufs=2))

            thunks2 = [lambda tp=tp, nn=nn: out_half1(tp, nn)
                       for tp in range(TOK // P) for nn in range(2)]
            ti = 0
            for h in range(11, H):
                ti = attention_head(h, thunks2, ti, thunk_slots=(1, 3, 5))
            while ti < len(thunks2):
                thunks2[ti]()
                ti += 1

            if dbg is not None:
                nc.sync.dma_start(out=dbg["d_kT"], in_=kz)
                nc.sync.dma_start(out=dbg["d_qT"], in_=qT)
                nc.sync.dma_start(out=dbg["d_yT"], in_=yT)

            # ---- second-half contraction + staged first half -> out
            for tp in range(TOK // P):
                o_sb = pO.tile([P, C], F32, tag="o", name="o_sb")
                for nn in range(C // TN):
                    ps = psP.tile([P, TN], F32, tag="pp", name="psO2")
                    for kc in range(NCH // 2, NCH):
                        nc.tensor.matmul(
                            ps, yT[:, kc, tp * P:(tp + 1) * P],
                            wo[:, kc, nn * TN:(nn + 1) * TN],
                            start=(kc == NCH // 2), stop=(kc == NCH - 1))
                    nc.vector.tensor_add(
                        o_sb[:, nn * TN:(nn + 1) * TN], ps,
                        state["o_part"][tp][:, nn * TN:(nn + 1) * TN])
                q = nc.sync if tp % 2 == 0 else nc.scalar
                q.dma_start(out=out[tp * P:(tp + 1) * P, :], in_=o_sb)


def make_in_maps(inputs):
    """Full fp32 inputs -> per-core input maps (host-side shard + layout)."""
    import ml_dtypes

    bf16 = ml_dtypes.bfloat16
    x = np.asarray(inputs["x"], dtype=np.float32)
    enc = np.asarray(inputs["enc_x"], dtype=np.float32)
    half = x.shape[1] // 2

    def chunked_T(a2d):
        # [rows, C] fp32 -> [128, NCH, rows] bf16 with [p, c, r] = a[r, c*128+p]
        t = np.ascontiguousarray(
            a2d.T.reshape(NCH, P, a2d.shape[0]).transpose(1, 0, 2))
        return t.astype(bf16)

    weights = {}
    for name, key in (("wq", "Wq"), ("wk", "Wk"), ("wv", "Wv"), ("wo", "Wo")):
        W = np.asarray(inputs[key], dtype=np.float32)
        weights[name] = np.ascontiguousarray(
            W.reshape(NCH, P, C).transpose(1, 0, 2)).astype(bf16)
    weights["bqc"] = np.ascontiguousarray(
        np.asarray(inputs["bq"], np.float32).reshape(NCH, P).T)
    weights["bkc"] = np.ascontiguousarray(
        np.asarray(inputs["bk"], np.float32).reshape(NCH, P).T)
    weights["bv"] = np.asarray(inputs["bv"], np.float32)
    weights["bo"] = np.asarray(inputs["bo"], np.float32)

    encT = [chunked_T(enc[b]) for b in range(x.shape[0])]
    maps = []
    for core in range(N_CORES):
        b, th = core // 2, core % 2
        m = {"xT": chunked_T(x[b, th * half:(th + 1) * half, :]),
             "encT": encT[b]}
        m.update(weights)
        maps.append(m)
    return maps


_CACHED = None


def _get_program():
    global _CACHED
    if _CACHED is None:
        _CACHED = build_program()
    return _CACHED


def kernel(**inputs):
    x = np.asarray(inputs["x"], dtype=np.float32)
    B, T, Cx = x.shape
    assert (B, T, Cx) == (B_FULL, T_FULL, C), (B, T, Cx)
    half = T // 2

    nc = _get_program()
    in_maps = make_in_maps(inputs)

    from concourse.bass_utils import run_bass_kernel_spmd
    res = None
    last_err = None
    for _attempt in range(3):
        try:
            res = run_bass_kernel_spmd(nc, in_maps,
                                       core_ids=list(range(N_CORES)))
            break
        except Exception as e:  # transient NRT/axon failures: retry
            last_err = e
    if res is None:
        raise last_err

    outp = np.empty((B, T, C), dtype=np.float32)
    for core in range(N_CORES):
        b, th = core // 2, core % 2
        outp[b, th * half:(th + 1) * half, :] = res.results[core]["out"]
    return outp


if __name__ == "__main__":
    prog = build_program()
    n_inst = sum(len(blk.instructions) for fn in prog.m.functions
                 for blk in fn.blocks)
    print("built OK; instructions:", n_inst)
